# revision 15
# baseline (speedup 1.0000x reference)
"""Trainium2 Bass kernel for nn_AdditiveAttention (Bahdanau attention).

Reference computation (B=16, Q=128, K=128, D=512, H=512):
    q = queries @ Wq                     [B,Q,H]
    k = keys @ Wk                        [B,K,H]
    scores[b,q,k] = sum_h wv[h] * tanh(q[b,q,h] + k[b,k,h])
    attn = softmax over valid keys (k < valid_lens[b])
    out = attn @ values                  [B,Q,D]

Strategy (8 NeuronCores, SPMD data-parallel):
  Work is proportional to (#queries x valid_len) per batch, and queries are
  embarrassingly parallel (each query's softmax is independent).  Each batch
  is split into q-range fragments; fragments are sorted by valid_len and
  distributed over 8 cores x S uniform "slots" (one SPMD program).  Slot j
  has a fixed shape (Qs_j, V_j) = (fragment q-size, max valid_len in that
  slot across cores); shorter fragments are handled with an additive -1e9
  key mask.  Sorted assignment keeps slot V_j close to each member's
  valid_len, minimizing padded work.

  On-device per slot: project queriesT/keysT transposed ([h=partitions]),
  per key-column k: pre[h,q] = q_projT + k_col broadcast-add on DVE (grouped
  stride-0 tensor_tensor), tanh on ScalarE in big batched instructions, wv
  reduction on TensorE (lhsT = tanh tile [128h,Qs], rhs = wv chunk [128h,1]
  -> scores column, one independent matmul per (k, h-chunk) into 4
  per-chunk PSUM banks), bank-sum + mask on DVE, exp with fused accumulated
  sum on ScalarE, reciprocal on DVE, transpose of the exp matrix on
  TensorE, and a final attn @ values matmul scaled by 1/z.
  bf16 on PE/DVE with fp32 PSUM accumulation; tanh/exp are fp32 internally.
"""

import os
import sys
import types
import math
import numpy as np
import ml_dtypes

# ---------------------------------------------------------------------------
# axon NTFF profile hook (lets trace=True / BASS_TRACE=1 work in this image)
# ---------------------------------------------------------------------------
def _install_axon_hooks():
    if "antenv.axon_hooks" in sys.modules:
        return
    try:
        import trn_agent_boot.trn_boot as _tb

        _hooks = types.ModuleType("antenv.axon_hooks")
        _hook = _tb._ntff_profile_via_ctypes("/opt/axon/libaxon_pjrt.so")
        _hooks.get_axon_ntff_profile_hook = lambda: _hook
        _hooks.set_axon_ntff_profile_hook = lambda h: None
        sys.modules["antenv.axon_hooks"] = _hooks
    except Exception:
        pass


_install_axon_hooks()

import concourse.bass as bass
import concourse.bacc as bacc
import concourse.mybir as mybir
import concourse.tile as tile
import concourse.bass_utils as bass_utils
from concourse.bass_utils import run_bass_kernel_spmd
from concourse.masks import make_identity

# Avoid S3 artifact-upload attempts in the trace path.
bass_utils.upload_artifacts = lambda tmpdir: tmpdir

F32 = mybir.dt.float32
BF16 = mybir.dt.bfloat16
BF16_NP = ml_dtypes.bfloat16

B, Q, K, D, H = 16, 128, 128, 512, 512
NCORES = 8
KT = 32  # key-columns per tanh group
NEG = -1e9

_NC_CACHE: dict = {}
LAST_RESULT = None


def _plan(valid_lens):
    """Pick fragmentation scheme; return (slots, content).

    slots:   tuple of (Qs, V) uniform slot shapes
    content: per-core list of fragments (batch, q0, qs, v), one per slot
    """
    best = None
    for nsplit in (1, 2):
        S = (B * nsplit) // NCORES
        qs = Q // nsplit
        frags = [
            (b, i * qs, qs, int(valid_lens[b]))
            for b in range(B)
            for i in range(nsplit)
        ]
        frags.sort(key=lambda f: -f[3])
        slots = []
        content = [[] for _ in range(NCORES)]
        cost = 0.0
        for j in range(S):
            grp = frags[j * NCORES : (j + 1) * NCORES]
            V = max(f[3] for f in grp)
            slots.append((qs, V))
            cost += qs * V
            for c, f in enumerate(grp):
                content[c].append(f)
        cost += S * 600.0  # per-slot fixed overhead, in q*k units
        if best is None or cost < best[0]:
            best = (cost, tuple(slots), content)
    return best[1], best[2]


def _build_nc(slots):
    """Build + finalize the single-core SPMD program for the given slots."""
    S = len(slots)
    maxQs = max(q for q, _ in slots)
    nc = bacc.Bacc(None, target_bir_lowering=False, debug=False)

    qT = nc.declare_dram_parameter("qT", [S, D, 128], BF16, isOutput=False)
    kT = nc.declare_dram_parameter("kT", [S, D, 128], BF16, isOutput=False)
    vals = nc.declare_dram_parameter("vals", [S, K, D], BF16, isOutput=False)
    wq_d = nc.declare_dram_parameter("wq", [D, H], BF16, isOutput=False)
    wk_d = nc.declare_dram_parameter("wk", [D, H], BF16, isOutput=False)
    wv_d = nc.declare_dram_parameter("wv4", [128, 4], BF16, isOutput=False)
    mask_d = nc.declare_dram_parameter("mask", [S, 128, K], F32, isOutput=False)
    out_d = nc.declare_dram_parameter("out", [S, 128, D], F32, isOutput=True)

    Tanh = mybir.ActivationFunctionType.Tanh
    Exp = mybir.ActivationFunctionType.Exp
    stage_bufs = 3 if maxQs <= 64 else 2

    with tile.TileContext(nc) as tc:
        with (
            tc.tile_pool(name="const", bufs=1) as constp,
            tc.tile_pool(name="io", bufs=1) as iop,
            tc.tile_pool(name="proj", bufs=1) as projp,
            tc.tile_pool(name="stage", bufs=stage_bufs) as stagep,
            tc.tile_pool(name="sm", bufs=2) as smp,
            tc.tile_pool(name="ps_proj", bufs=2, space="PSUM") as ps_proj,
            tc.tile_pool(name="ps_sc", bufs=4, space="PSUM") as ps_sc,
            tc.tile_pool(name="ps_misc", bufs=1, space="PSUM") as ps_misc,
        ):
            # ---- constants & inputs -------------------------------------
            wq_sb = constp.tile([128, 4, H], BF16, tag="wq")
            nc.sync.dma_start(wq_sb[:], wq_d[:].rearrange("(c p) h -> p c h", p=128))
            wk_sb = constp.tile([128, 4, H], BF16, tag="wk")
            nc.sync.dma_start(wk_sb[:], wk_d[:].rearrange("(c p) h -> p c h", p=128))
            qt_sb = iop.tile([128, S, 4, 128], BF16, tag="qt")
            nc.sync.dma_start(qt_sb[:], qT[:].rearrange("s (c p) q -> p s c q", p=128))
            kt_sb = iop.tile([128, S, 4, 128], BF16, tag="kt")
            nc.sync.dma_start(kt_sb[:], kT[:].rearrange("s (c p) k -> p s c k", p=128))
            wv_sb = constp.tile([128, 4], BF16, tag="wv")
            nc.sync.dma_start(wv_sb[:], wv_d[:])
            ident = constp.tile([128, 128], BF16, tag="ident")
            make_identity(nc, ident[:])
            vals_sb = iop.tile([128, S, D], BF16, tag="vals")
            nc.sync.dma_start(vals_sb[:], vals[:].rearrange("s k d -> k s d"))
            mask_sb = iop.tile([128, S, K], F32, tag="mask")
            nc.sync.dma_start(mask_sb[:], mask_d[:].rearrange("s p k -> p s k"))

            # ---- projections: projT[h,x] = sum_d W[d,h] * xT[d,x] -------
            qproj = projp.tile([128, S, 4, 128], BF16, tag="qproj")
            kproj = projp.tile([128, S, 4, 128], BF16, tag="kproj")
            for s in range(S):
                Qs, V = slots[s]
                for hc in range(4):
                    pq = ps_proj.tile([128, 128], F32, tag="pp")
                    for dc in range(4):
                        nc.tensor.matmul(
                            pq[:, :Qs],
                            wq_sb[:, dc, hc * 128 : (hc + 1) * 128],
                            qt_sb[:, s, dc, :Qs],
                            start=(dc == 0),
                            stop=(dc == 3),
                        )
                    nc.scalar.copy(qproj[:, s, hc, :Qs], pq[:, :Qs])
                    pk = ps_proj.tile([128, 128], F32, tag="pp")
                    for dc in range(4):
                        nc.tensor.matmul(
                            pk[:, :V],
                            wk_sb[:, dc, hc * 128 : (hc + 1) * 128],
                            kt_sb[:, s, dc, :V],
                            start=(dc == 0),
                            stop=(dc == 3),
                        )
                    nc.scalar.copy(kproj[:, s, hc, :V], pk[:, :V])

            # persistent softmax state
            e_sb = projp.tile([128, S, 128], BF16, tag="e")
            nc.vector.memset(e_sb[:], 0.0)
            rz = projp.tile([128, S], F32, tag="rz")

            # ---- main loop ----------------------------------------------
            for s in range(S):
                Qs, V = slots[s]
                ngroups = math.ceil(V / KT)
                # single score bank per slot: per key, 4 h-chunk matmuls
                # accumulate into one column; PE issue order pinned so a
                # start=True (whole-bank has_written clear) never lands
                # inside another key's accumulation group.
                psc = ps_sc.tile([128, K], F32, tag="psc")
                prev_last = None
                for g in range(ngroups):
                    k0 = g * KT
                    Kg = min(KT, V - k0)
                    pre = stagep.tile([128, 4, KT, maxQs], BF16, tag="pre")
                    tnh = stagep.tile([128, 4, KT, maxQs], BF16, tag="tnh")
                    for hc in range(4):
                        # pre[h, kl, q] = kproj[h, k0+kl] + qproj[h, q]
                        in0 = (
                            kproj[:, s, hc, k0 : k0 + Kg]
                            .unsqueeze(2)
                            .broadcast_to((128, Kg, Qs))
                        )
                        in1 = (
                            qproj[:, s, hc, :Qs]
                            .unsqueeze(1)
                            .broadcast_to((128, Kg, Qs))
                        )
                        nc.vector.tensor_add(pre[:, hc, :Kg, :Qs], in0, in1)
                    nc.scalar.activation(
                        tnh[:, 0:2, :Kg, :Qs], pre[:, 0:2, :Kg, :Qs], Tanh
                    )
                    nc.scalar.activation(
                        tnh[:, 2:4, :Kg, :Qs], pre[:, 2:4, :Kg, :Qs], Tanh
                    )
                    for kl in range(Kg):
                        first = None
                        for hc in range(4):
                            bi = nc.tensor.matmul(
                                psc[:Qs, k0 + kl : k0 + kl + 1],
                                tnh[:, hc, kl, :Qs],
                                wv_sb[:, hc : hc + 1],
                                start=(hc == 0),
                                stop=(hc == 3),
                            )
                            if hc == 0:
                                first = bi.ins
                            last = bi.ins
                        if prev_last is not None:
                            tile.add_dep_helper(
                                first, prev_last, sync=False,
                                reason="psc accumulation-group order",
                            )
                        prev_last = last
                # ---- softmax over the V columns -------------------------
                msc = smp.tile([128, K], F32, tag="msc")
                nc.vector.tensor_add(
                    msc[:Qs, :V], psc[:Qs, :V], mask_sb[:Qs, s, :V]
                )
                z = smp.tile([128, 1], F32, tag="z")
                nc.scalar.activation(
                    e_sb[:Qs, s, :V], msc[:Qs, :V], Exp, accum_out=z[:Qs, :]
                )
                nc.vector.reciprocal(rz[:Qs, s : s + 1], z[:Qs, :])
                # ---- attn @ values --------------------------------------
                pt = ps_misc.tile([128, 128], BF16, tag="pt")
                nc.tensor.transpose(pt[:], e_sb[:, s, :], ident[:])
                eT = smp.tile([128, 128], BF16, tag="eT")
                nc.vector.tensor_copy(eT[:], pt[:])
                po = ps_misc.tile([128, D], F32, tag="po")
                nc.tensor.matmul(
                    po[:, :], eT[:V, :], vals_sb[:V, s, :], start=True, stop=True
                )
                o_sb = smp.tile([128, D], F32, tag="o")
                nc.vector.tensor_scalar_mul(
                    o_sb[:Qs, :], po[:Qs, :], rz[:Qs, s : s + 1]
                )
                nc.sync.dma_start(out_d[s, :Qs], o_sb[:Qs, :])

    nc.finalize()
    return nc


def kernel(queries, keys, values, valid_lens, Wq, Wk, wv):
    global LAST_RESULT
    queries = np.asarray(queries, dtype=np.float32)
    keys = np.asarray(keys, dtype=np.float32)
    values = np.asarray(values, dtype=np.float32)
    valid_lens = np.asarray(valid_lens, dtype=np.int32)
    Wq = np.asarray(Wq, dtype=np.float32)
    Wk = np.asarray(Wk, dtype=np.float32)
    wv = np.asarray(wv, dtype=np.float32)

    slots, content = _plan(valid_lens)
    S = len(slots)

    if slots not in _NC_CACHE:
        _NC_CACHE[slots] = _build_nc(slots)
    nc = _NC_CACHE[slots]

    # ---- host-side shard prep -------------------------------------------
    wq_bf = Wq.astype(BF16_NP)
    wk_bf = Wk.astype(BF16_NP)
    wv4 = np.ascontiguousarray(wv.reshape(4, 128).T).astype(BF16_NP)  # [128,4]

    in_maps = []
    for c in range(NCORES):
        qTm = np.zeros((S, D, 128), dtype=BF16_NP)
        kTm = np.zeros((S, D, 128), dtype=BF16_NP)
        valsm = np.zeros((S, K, D), dtype=BF16_NP)
        maskm = np.zeros((S, 128, K), dtype=np.float32)
        for s, (b, q0, qs, v) in enumerate(content[c]):
            qTm[s, :, :qs] = queries[b, q0 : q0 + qs].T.astype(BF16_NP)
            kTm[s, :, :v] = keys[b, :v].T.astype(BF16_NP)
            valsm[s, :v] = values[b, :v].astype(BF16_NP)
            maskm[s, :, v:] = NEG
        in_maps.append(
            {
                "qT": qTm,
                "kT": kTm,
                "vals": valsm,
                "wq": wq_bf,
                "wk": wk_bf,
                "wv4": wv4,
                "mask": maskm,
            }
        )

    res = run_bass_kernel_spmd(nc, in_maps, list(range(NCORES)))
    LAST_RESULT = res

    out = np.empty((B, Q, D), dtype=np.float32)
    for c in range(NCORES):
        o = np.asarray(res.results[c]["out"], dtype=np.float32)
        for s, (b, q0, qs, v) in enumerate(content[c]):
            out[b, q0 : q0 + qs] = o[s, :qs]
    return out


# revision 17
# speedup vs baseline: 1.0266x; 1.0266x over previous
"""Trainium2 Bass kernel for nn_AdditiveAttention (Bahdanau attention).

Reference computation (B=16, Q=128, K=128, D=512, H=512):
    q = queries @ Wq                     [B,Q,H]
    k = keys @ Wk                        [B,K,H]
    scores[b,q,k] = sum_h wv[h] * tanh(q[b,q,h] + k[b,k,h])
    attn = softmax over valid keys (k < valid_lens[b])
    out = attn @ values                  [B,Q,D]

Strategy (8 NeuronCores, SPMD data-parallel):
  Work is proportional to (#queries x valid_len) per batch, and queries are
  embarrassingly parallel (each query's softmax is independent).  Each batch
  is split into q-range fragments; fragments are sorted by valid_len and
  distributed over 8 cores x S uniform "slots" (one SPMD program).  Slot j
  has a fixed shape (Qs_j, V_j) = (fragment q-size, max valid_len in that
  slot across cores); shorter fragments are handled with an additive -1e9
  key mask.  Sorted assignment keeps slot V_j close to each member's
  valid_len, minimizing padded work.

  On-device per slot: project queriesT/keysT transposed ([h=partitions]),
  per key-column k: pre[h,q] = q_projT + k_col broadcast-add on DVE (grouped
  stride-0 tensor_tensor), tanh on ScalarE in big batched instructions, wv
  reduction on TensorE (lhsT = tanh tile [128h,Qs], rhs = wv chunk [128h,1]
  -> scores column, one independent matmul per (k, h-chunk) into 4
  per-chunk PSUM banks), bank-sum + mask on DVE, exp with fused accumulated
  sum on ScalarE, reciprocal on DVE, transpose of the exp matrix on
  TensorE, and a final attn @ values matmul scaled by 1/z.
  bf16 on PE/DVE with fp32 PSUM accumulation; tanh/exp are fp32 internally.
"""

import os
import sys
import types
import math
import numpy as np
import ml_dtypes

# ---------------------------------------------------------------------------
# axon NTFF profile hook (lets trace=True / BASS_TRACE=1 work in this image)
# ---------------------------------------------------------------------------
def _install_axon_hooks():
    if "antenv.axon_hooks" in sys.modules:
        return
    try:
        import trn_agent_boot.trn_boot as _tb

        _hooks = types.ModuleType("antenv.axon_hooks")
        _hook = _tb._ntff_profile_via_ctypes("/opt/axon/libaxon_pjrt.so")
        _hooks.get_axon_ntff_profile_hook = lambda: _hook
        _hooks.set_axon_ntff_profile_hook = lambda h: None
        sys.modules["antenv.axon_hooks"] = _hooks
    except Exception:
        pass


_install_axon_hooks()

import concourse.bass as bass
import concourse.bacc as bacc
import concourse.mybir as mybir
import concourse.tile as tile
import concourse.bass_utils as bass_utils
from concourse.bass_utils import run_bass_kernel_spmd
from concourse.masks import make_identity

# Avoid S3 artifact-upload attempts in the trace path.
bass_utils.upload_artifacts = lambda tmpdir: tmpdir

F32 = mybir.dt.float32
BF16 = mybir.dt.bfloat16
BF16_NP = ml_dtypes.bfloat16

B, Q, K, D, H = 16, 128, 128, 512, 512
NCORES = 8
KT = 32  # key-columns per tanh group
NEG = -1e9

_NC_CACHE: dict = {}
LAST_RESULT = None


def _plan(valid_lens):
    """Pick fragmentation scheme; return (slots, content).

    slots:   tuple of (Qs, V) uniform slot shapes
    content: per-core list of fragments (batch, q0, qs, v), one per slot
    """
    best = None
    for nsplit in (1, 2):
        S = (B * nsplit) // NCORES
        qs = Q // nsplit
        frags = [
            (b, i * qs, qs, int(valid_lens[b]))
            for b in range(B)
            for i in range(nsplit)
        ]
        frags.sort(key=lambda f: -f[3])
        slots = []
        content = [[] for _ in range(NCORES)]
        cost = 0.0
        for j in range(S):
            grp = frags[j * NCORES : (j + 1) * NCORES]
            V = max(f[3] for f in grp)
            slots.append((qs, V))
            cost += qs * V
            for c, f in enumerate(grp):
                content[c].append(f)
        cost += S * 600.0  # per-slot fixed overhead, in q*k units
        if best is None or cost < best[0]:
            best = (cost, tuple(slots), content)
    return best[1], best[2]


def _build_nc(slots):
    """Build + finalize the single-core SPMD program for the given slots."""
    S = len(slots)
    maxQs = max(q for q, _ in slots)
    nc = bacc.Bacc(None, target_bir_lowering=False, debug=False)

    qT = nc.declare_dram_parameter("qT", [S, D, 128], BF16, isOutput=False)
    kT = nc.declare_dram_parameter("kT", [S, D, 128], BF16, isOutput=False)
    vals = nc.declare_dram_parameter("vals", [S, K, D], BF16, isOutput=False)
    wq_d = nc.declare_dram_parameter("wq", [D, H], BF16, isOutput=False)
    wk_d = nc.declare_dram_parameter("wk", [D, H], BF16, isOutput=False)
    wv_d = nc.declare_dram_parameter("wv4", [128, 4], BF16, isOutput=False)
    mask_d = nc.declare_dram_parameter("mask", [S, 128, K], F32, isOutput=False)
    out_d = nc.declare_dram_parameter("out", [S, 128, D], F32, isOutput=True)

    Tanh = mybir.ActivationFunctionType.Tanh
    Exp = mybir.ActivationFunctionType.Exp
    stage_bufs = 3 if maxQs <= 64 else 2

    with tile.TileContext(nc) as tc:
        with (
            tc.tile_pool(name="const", bufs=1) as constp,
            tc.tile_pool(name="io", bufs=1) as iop,
            tc.tile_pool(name="proj", bufs=1) as projp,
            tc.tile_pool(name="stage", bufs=stage_bufs) as stagep,
            tc.tile_pool(name="sm", bufs=2) as smp,
            tc.tile_pool(name="ps_proj", bufs=2, space="PSUM") as ps_proj,
            tc.tile_pool(name="ps_sc", bufs=4, space="PSUM") as ps_sc,
            tc.tile_pool(name="ps_misc", bufs=1, space="PSUM") as ps_misc,
        ):
            # ---- constants & inputs -------------------------------------
            wq_sb = constp.tile([128, 4, H], BF16, tag="wq")
            nc.sync.dma_start(wq_sb[:], wq_d[:].rearrange("(c p) h -> p c h", p=128))
            wk_sb = constp.tile([128, 4, H], BF16, tag="wk")
            nc.sync.dma_start(wk_sb[:], wk_d[:].rearrange("(c p) h -> p c h", p=128))
            qt_sb = iop.tile([128, S, 4, 128], BF16, tag="qt")
            nc.sync.dma_start(qt_sb[:], qT[:].rearrange("s (c p) q -> p s c q", p=128))
            kt_sb = iop.tile([128, S, 4, 128], BF16, tag="kt")
            nc.sync.dma_start(kt_sb[:], kT[:].rearrange("s (c p) k -> p s c k", p=128))
            wv_sb = constp.tile([128, 4], BF16, tag="wv")
            nc.sync.dma_start(wv_sb[:], wv_d[:])
            ident = constp.tile([128, 128], BF16, tag="ident")
            make_identity(nc, ident[:])
            vals_sb = iop.tile([128, S, D], BF16, tag="vals")
            nc.sync.dma_start(vals_sb[:], vals[:].rearrange("s k d -> k s d"))
            mask_sb = iop.tile([128, S, K], F32, tag="mask")
            nc.sync.dma_start(mask_sb[:], mask_d[:].rearrange("s p k -> p s k"))

            # ---- projections: projT[h,x] = sum_d W[d,h] * xT[d,x] -------
            qproj = projp.tile([128, S, 4, 128], BF16, tag="qproj")
            kproj = projp.tile([128, S, 4, 128], BF16, tag="kproj")
            for s in range(S):
                Qs, V = slots[s]
                for hc in range(4):
                    pq = ps_proj.tile([128, 128], F32, tag="pp")
                    for dc in range(4):
                        nc.tensor.matmul(
                            pq[:, :Qs],
                            wq_sb[:, dc, hc * 128 : (hc + 1) * 128],
                            qt_sb[:, s, dc, :Qs],
                            start=(dc == 0),
                            stop=(dc == 3),
                        )
                    nc.scalar.copy(qproj[:, s, hc, :Qs], pq[:, :Qs])
                    pk = ps_proj.tile([128, 128], F32, tag="pp")
                    for dc in range(4):
                        nc.tensor.matmul(
                            pk[:, :V],
                            wk_sb[:, dc, hc * 128 : (hc + 1) * 128],
                            kt_sb[:, s, dc, :V],
                            start=(dc == 0),
                            stop=(dc == 3),
                        )
                    nc.scalar.copy(kproj[:, s, hc, :V], pk[:, :V])

            # persistent softmax state
            e_sb = projp.tile([128, S, 128], BF16, tag="e")
            nc.vector.memset(e_sb[:], 0.0)
            rz = projp.tile([128, S], F32, tag="rz")

            # ---- main loop ----------------------------------------------
            # Slot epilogues (softmax + output) are emitted one slot late:
            # engines are in-order, so emitting an epilogue (which waits on
            # the slot's full PE matmul tail) before the next slot's group
            # work would head-of-line-block every engine at each slot
            # boundary.
            def epilogue(s, psc):
                Qs, V = slots[s]
                msc = smp.tile([128, K], F32, tag="msc", name=f"msc{s}")
                nc.vector.tensor_add(
                    msc[:Qs, :V], psc[:Qs, :V], mask_sb[:Qs, s, :V]
                )
                z = smp.tile([128, 1], F32, tag="z", name=f"z{s}")
                nc.scalar.activation(
                    e_sb[:Qs, s, :V], msc[:Qs, :V], Exp, accum_out=z[:Qs, :]
                )
                nc.vector.reciprocal(rz[:Qs, s : s + 1], z[:Qs, :])
                pt = ps_misc.tile([128, 128], BF16, tag="pt", name=f"pt{s}")
                nc.tensor.transpose(pt[:], e_sb[:, s, :], ident[:])
                eT = smp.tile([128, 128], BF16, tag="eT", name=f"eT{s}")
                nc.vector.tensor_copy(eT[:], pt[:])
                po = ps_misc.tile([128, D], F32, tag="po", name=f"po{s}")
                nc.tensor.matmul(
                    po[:, :], eT[:V, :], vals_sb[:V, s, :], start=True, stop=True
                )
                o_sb = smp.tile([128, D], F32, tag="o", name=f"o{s}")
                nc.vector.tensor_scalar_mul(
                    o_sb[:Qs, :], po[:Qs, :], rz[:Qs, s : s + 1]
                )
                nc.sync.dma_start(out_d[s, :Qs], o_sb[:Qs, :])

            pending = None  # (s, psc) awaiting epilogue
            for s in range(S):
                Qs, V = slots[s]
                ngroups = math.ceil(V / KT)
                # single score bank per slot: per key, 4 h-chunk matmuls
                # accumulate into one column; PE issue order pinned so a
                # start=True (whole-bank has_written clear) never lands
                # inside another key's accumulation group.
                psc = ps_sc.tile([128, K], F32, tag="psc")
                prev_last = None
                for g in range(ngroups):
                    k0 = g * KT
                    Kg = min(KT, V - k0)
                    pre = stagep.tile([128, 4, KT, maxQs], BF16, tag="pre")
                    tnh = stagep.tile([128, 4, KT, maxQs], BF16, tag="tnh")
                    for hc in range(4):
                        # pre[h, kl, q] = kproj[h, k0+kl] + qproj[h, q]
                        in0 = (
                            kproj[:, s, hc, k0 : k0 + Kg]
                            .unsqueeze(2)
                            .broadcast_to((128, Kg, Qs))
                        )
                        in1 = (
                            qproj[:, s, hc, :Qs]
                            .unsqueeze(1)
                            .broadcast_to((128, Kg, Qs))
                        )
                        nc.vector.tensor_add(pre[:, hc, :Kg, :Qs], in0, in1)
                    nc.scalar.activation(
                        tnh[:, 0:2, :Kg, :Qs], pre[:, 0:2, :Kg, :Qs], Tanh
                    )
                    nc.scalar.activation(
                        tnh[:, 2:4, :Kg, :Qs], pre[:, 2:4, :Kg, :Qs], Tanh
                    )
                    for kl in range(Kg):
                        first = None
                        for hc in range(4):
                            bi = nc.tensor.matmul(
                                psc[:Qs, k0 + kl : k0 + kl + 1],
                                tnh[:, hc, kl, :Qs],
                                wv_sb[:, hc : hc + 1],
                                start=(hc == 0),
                                stop=(hc == 3),
                            )
                            if hc == 0:
                                first = bi.ins
                            last = bi.ins
                        if prev_last is not None:
                            tile.add_dep_helper(
                                first, prev_last, sync=False,
                                reason="psc accumulation-group order",
                            )
                        prev_last = last
                    if g == 0 and pending is not None:
                        epilogue(*pending)
                        pending = None
                pending = (s, psc)
            epilogue(*pending)

    nc.finalize()
    return nc


def kernel(queries, keys, values, valid_lens, Wq, Wk, wv):
    global LAST_RESULT
    queries = np.asarray(queries, dtype=np.float32)
    keys = np.asarray(keys, dtype=np.float32)
    values = np.asarray(values, dtype=np.float32)
    valid_lens = np.asarray(valid_lens, dtype=np.int32)
    Wq = np.asarray(Wq, dtype=np.float32)
    Wk = np.asarray(Wk, dtype=np.float32)
    wv = np.asarray(wv, dtype=np.float32)

    slots, content = _plan(valid_lens)
    S = len(slots)

    if slots not in _NC_CACHE:
        _NC_CACHE[slots] = _build_nc(slots)
    nc = _NC_CACHE[slots]

    # ---- host-side shard prep -------------------------------------------
    wq_bf = Wq.astype(BF16_NP)
    wk_bf = Wk.astype(BF16_NP)
    wv4 = np.ascontiguousarray(wv.reshape(4, 128).T).astype(BF16_NP)  # [128,4]

    in_maps = []
    for c in range(NCORES):
        qTm = np.zeros((S, D, 128), dtype=BF16_NP)
        kTm = np.zeros((S, D, 128), dtype=BF16_NP)
        valsm = np.zeros((S, K, D), dtype=BF16_NP)
        maskm = np.zeros((S, 128, K), dtype=np.float32)
        for s, (b, q0, qs, v) in enumerate(content[c]):
            qTm[s, :, :qs] = queries[b, q0 : q0 + qs].T.astype(BF16_NP)
            kTm[s, :, :v] = keys[b, :v].T.astype(BF16_NP)
            valsm[s, :v] = values[b, :v].astype(BF16_NP)
            maskm[s, :, v:] = NEG
        in_maps.append(
            {
                "qT": qTm,
                "kT": kTm,
                "vals": valsm,
                "wq": wq_bf,
                "wk": wk_bf,
                "wv4": wv4,
                "mask": maskm,
            }
        )

    res = run_bass_kernel_spmd(nc, in_maps, list(range(NCORES)))
    LAST_RESULT = res

    out = np.empty((B, Q, D), dtype=np.float32)
    for c in range(NCORES):
        o = np.asarray(res.results[c]["out"], dtype=np.float32)
        for s, (b, q0, qs, v) in enumerate(content[c]):
            out[b, q0 : q0 + qs] = o[s, :qs]
    return out


# revision 19
# speedup vs baseline: 1.0552x; 1.0278x over previous
"""Trainium2 Bass kernel for nn_AdditiveAttention (Bahdanau attention).

Reference computation (B=16, Q=128, K=128, D=512, H=512):
    q = queries @ Wq                     [B,Q,H]
    k = keys @ Wk                        [B,K,H]
    scores[b,q,k] = sum_h wv[h] * tanh(q[b,q,h] + k[b,k,h])
    attn = softmax over valid keys (k < valid_lens[b])
    out = attn @ values                  [B,Q,D]

Strategy (8 NeuronCores, SPMD data-parallel):
  Work is proportional to (#queries x valid_len) per batch, and queries are
  embarrassingly parallel (each query's softmax is independent).  Each batch
  is split into q-range fragments; fragments are sorted by valid_len and
  distributed over 8 cores x S uniform "slots" (one SPMD program).  Slot j
  has a fixed shape (Qs_j, V_j) = (fragment q-size, max valid_len in that
  slot across cores); shorter fragments are handled with an additive -1e9
  key mask.  Sorted assignment keeps slot V_j close to each member's
  valid_len, minimizing padded work.

  On-device per slot: project queriesT/keysT transposed ([h=partitions]),
  per key-column k: pre[h,q] = q_projT + k_col broadcast-add on DVE (grouped
  stride-0 tensor_tensor), tanh on ScalarE in big batched instructions, wv
  reduction on TensorE (lhsT = tanh tile [128h,Qs], rhs = wv chunk [128h,1]
  -> scores column, one independent matmul per (k, h-chunk) into 4
  per-chunk PSUM banks), bank-sum + mask on DVE, exp with fused accumulated
  sum on ScalarE, reciprocal on DVE, transpose of the exp matrix on
  TensorE, and a final attn @ values matmul scaled by 1/z.
  bf16 on PE/DVE with fp32 PSUM accumulation; tanh/exp are fp32 internally.
"""

import os
import sys
import types
import math
import numpy as np
import ml_dtypes

# ---------------------------------------------------------------------------
# axon NTFF profile hook (lets trace=True / BASS_TRACE=1 work in this image)
# ---------------------------------------------------------------------------
def _install_axon_hooks():
    if "antenv.axon_hooks" in sys.modules:
        return
    try:
        import trn_agent_boot.trn_boot as _tb

        _hooks = types.ModuleType("antenv.axon_hooks")
        _hook = _tb._ntff_profile_via_ctypes("/opt/axon/libaxon_pjrt.so")
        _hooks.get_axon_ntff_profile_hook = lambda: _hook
        _hooks.set_axon_ntff_profile_hook = lambda h: None
        sys.modules["antenv.axon_hooks"] = _hooks
    except Exception:
        pass


_install_axon_hooks()

import concourse.bass as bass
import concourse.bacc as bacc
import concourse.mybir as mybir
import concourse.tile as tile
import concourse.bass_utils as bass_utils
from concourse.bass_utils import run_bass_kernel_spmd
from concourse.masks import make_identity

# Avoid S3 artifact-upload attempts in the trace path.
bass_utils.upload_artifacts = lambda tmpdir: tmpdir

F32 = mybir.dt.float32
BF16 = mybir.dt.bfloat16
BF16_NP = ml_dtypes.bfloat16

B, Q, K, D, H = 16, 128, 128, 512, 512
NCORES = 8
KT = 32  # key-columns per tanh group
NEG = -1e9

_NC_CACHE: dict = {}
LAST_RESULT = None


def _plan(valid_lens):
    """Pick fragmentation scheme; return (slots, content).

    slots:   tuple of (Qs, V) uniform slot shapes
    content: per-core list of fragments (batch, q0, qs, v), one per slot
    """
    best = None
    for nsplit in (1, 2):
        S = (B * nsplit) // NCORES
        qs = Q // nsplit
        frags = [
            (b, i * qs, qs, int(valid_lens[b]))
            for b in range(B)
            for i in range(nsplit)
        ]
        frags.sort(key=lambda f: -f[3])
        slots = []
        content = [[] for _ in range(NCORES)]
        cost = 0.0
        for j in range(S):
            grp = frags[j * NCORES : (j + 1) * NCORES]
            V = max(f[3] for f in grp)
            slots.append((qs, V))
            cost += qs * V
            for c, f in enumerate(grp):
                content[c].append(f)
        cost += S * 600.0  # per-slot fixed overhead, in q*k units
        if best is None or cost < best[0]:
            best = (cost, tuple(slots), content)
    return best[1], best[2]


def _build_nc(slots):
    """Build + finalize the single-core SPMD program for the given slots."""
    S = len(slots)
    maxQs = max(q for q, _ in slots)
    nc = bacc.Bacc(None, target_bir_lowering=False, debug=False)

    qT = nc.declare_dram_parameter("qT", [S, D, 128], BF16, isOutput=False)
    kT = nc.declare_dram_parameter("kT", [S, D, 128], BF16, isOutput=False)
    vals = nc.declare_dram_parameter("vals", [S, K, D], BF16, isOutput=False)
    wq_d = nc.declare_dram_parameter("wq", [D, H], BF16, isOutput=False)
    wk_d = nc.declare_dram_parameter("wk", [D, H], BF16, isOutput=False)
    wv_d = nc.declare_dram_parameter("wv4", [128, 4], BF16, isOutput=False)
    mask_d = nc.declare_dram_parameter("mask", [S, 128, K], F32, isOutput=False)
    out_d = nc.declare_dram_parameter("out", [S, 128, D], F32, isOutput=True)

    Tanh = mybir.ActivationFunctionType.Tanh
    Exp = mybir.ActivationFunctionType.Exp
    stage_bufs = 3 if maxQs <= 64 else 2

    with tile.TileContext(nc) as tc:
        with (
            tc.tile_pool(name="const", bufs=1) as constp,
            tc.tile_pool(name="io", bufs=1) as iop,
            tc.tile_pool(name="proj", bufs=1) as projp,
            tc.tile_pool(name="stage", bufs=stage_bufs) as stagep,
            tc.tile_pool(name="sm", bufs=2) as smp,
            tc.tile_pool(name="ps_proj", bufs=2, space="PSUM") as ps_proj,
            tc.tile_pool(name="ps_sc", bufs=4, space="PSUM") as ps_sc,
            tc.tile_pool(name="ps_misc", bufs=1, space="PSUM") as ps_misc,
        ):
            # ---- constants & inputs -------------------------------------
            wq_sb = constp.tile([128, 4, H], BF16, tag="wq")
            nc.sync.dma_start(wq_sb[:], wq_d[:].rearrange("(c p) h -> p c h", p=128))
            wk_sb = constp.tile([128, 4, H], BF16, tag="wk")
            nc.sync.dma_start(wk_sb[:], wk_d[:].rearrange("(c p) h -> p c h", p=128))
            qt_sb = iop.tile([128, S, 4, 128], BF16, tag="qt")
            kt_sb = iop.tile([128, S, 4, 128], BF16, tag="kt")
            qT_r = qT[:].rearrange("s (c p) q -> p s c q", p=128)
            kT_r = kT[:].rearrange("s (c p) k -> p s c k", p=128)
            for s in range(S):
                nc.sync.dma_start(qt_sb[:, s], qT_r[:, s])
                nc.sync.dma_start(kt_sb[:, s], kT_r[:, s])
            wv_sb = constp.tile([128, 4], BF16, tag="wv")
            nc.sync.dma_start(wv_sb[:], wv_d[:])
            ident = constp.tile([128, 128], BF16, tag="ident")
            make_identity(nc, ident[:])
            vals_sb = iop.tile([128, S, D], BF16, tag="vals")
            nc.sync.dma_start(vals_sb[:], vals[:].rearrange("s k d -> k s d"))
            mask_sb = iop.tile([128, S, K], F32, tag="mask")
            nc.sync.dma_start(mask_sb[:], mask_d[:].rearrange("s p k -> p s k"))

            # ---- projections: projT[h,x] = sum_d W[d,h] * xT[d,x] -------
            qproj = projp.tile([128, S, 4, 128], BF16, tag="qproj")
            kproj = projp.tile([128, S, 4, 128], BF16, tag="kproj")
            for s in range(S):
                Qs, V = slots[s]
                for hc in range(4):
                    pq = ps_proj.tile([128, 128], F32, tag="pp")
                    for dc in range(4):
                        nc.tensor.matmul(
                            pq[:, :Qs],
                            wq_sb[:, dc, hc * 128 : (hc + 1) * 128],
                            qt_sb[:, s, dc, :Qs],
                            start=(dc == 0),
                            stop=(dc == 3),
                        )
                    nc.scalar.copy(qproj[:, s, hc, :Qs], pq[:, :Qs])
                    pk = ps_proj.tile([128, 128], F32, tag="pp")
                    for dc in range(4):
                        nc.tensor.matmul(
                            pk[:, :V],
                            wk_sb[:, dc, hc * 128 : (hc + 1) * 128],
                            kt_sb[:, s, dc, :V],
                            start=(dc == 0),
                            stop=(dc == 3),
                        )
                    nc.scalar.copy(kproj[:, s, hc, :V], pk[:, :V])

            # persistent softmax state
            e_sb = projp.tile([128, S, 128], BF16, tag="e")
            nc.vector.memset(e_sb[:], 0.0)
            rz = projp.tile([128, S], F32, tag="rz")

            # ---- main loop ----------------------------------------------
            # Slot epilogues (softmax + output) are emitted one slot late:
            # engines are in-order, so emitting an epilogue (which waits on
            # the slot's full PE matmul tail) before the next slot's group
            # work would head-of-line-block every engine at each slot
            # boundary.
            def epilogue(s, psc):
                Qs, V = slots[s]
                msc = smp.tile([128, K], F32, tag="msc", name=f"msc{s}")
                nc.vector.tensor_add(
                    msc[:Qs, :V], psc[:Qs, :V], mask_sb[:Qs, s, :V]
                )
                z = smp.tile([128, 1], F32, tag="z", name=f"z{s}")
                nc.scalar.activation(
                    e_sb[:Qs, s, :V], msc[:Qs, :V], Exp, accum_out=z[:Qs, :]
                )
                nc.vector.reciprocal(rz[:Qs, s : s + 1], z[:Qs, :])
                pt = ps_misc.tile([128, 128], BF16, tag="pt", name=f"pt{s}")
                nc.tensor.transpose(pt[:], e_sb[:, s, :], ident[:])
                eT = smp.tile([128, 128], BF16, tag="eT", name=f"eT{s}")
                nc.vector.tensor_copy(eT[:], pt[:])
                po = ps_misc.tile([128, D], F32, tag="po", name=f"po{s}")
                nc.tensor.matmul(
                    po[:, :], eT[:V, :], vals_sb[:V, s, :], start=True, stop=True
                )
                o_sb = smp.tile([128, D], F32, tag="o", name=f"o{s}")
                nc.vector.tensor_scalar_mul(
                    o_sb[:Qs, :], po[:Qs, :], rz[:Qs, s : s + 1]
                )
                nc.sync.dma_start(out_d[s, :Qs], o_sb[:Qs, :])

            pending = None  # (s, psc) awaiting epilogue
            for s in range(S):
                Qs, V = slots[s]
                ngroups = math.ceil(V / KT)
                # single score bank per slot: per key, 4 h-chunk matmuls
                # accumulate into one column; PE issue order pinned so a
                # start=True (whole-bank has_written clear) never lands
                # inside another key's accumulation group.
                psc = ps_sc.tile([128, K], F32, tag="psc")
                prev_last = None
                for g in range(ngroups):
                    k0 = g * KT
                    Kg = min(KT, V - k0)
                    pre = stagep.tile([128, 4, KT, maxQs], BF16, tag="pre")
                    tnh = stagep.tile([128, 4, KT, maxQs], BF16, tag="tnh")
                    for hc in range(4):
                        # pre[h, kl, q] = kproj[h, k0+kl] + qproj[h, q]
                        in0 = (
                            kproj[:, s, hc, k0 : k0 + Kg]
                            .unsqueeze(2)
                            .broadcast_to((128, Kg, Qs))
                        )
                        in1 = (
                            qproj[:, s, hc, :Qs]
                            .unsqueeze(1)
                            .broadcast_to((128, Kg, Qs))
                        )
                        nc.vector.tensor_add(pre[:, hc, :Kg, :Qs], in0, in1)
                    nc.scalar.activation(
                        tnh[:, 0:2, :Kg, :Qs], pre[:, 0:2, :Kg, :Qs], Tanh
                    )
                    nc.scalar.activation(
                        tnh[:, 2:4, :Kg, :Qs], pre[:, 2:4, :Kg, :Qs], Tanh
                    )
                    for kl in range(Kg):
                        first = None
                        for hc in range(4):
                            bi = nc.tensor.matmul(
                                psc[:Qs, k0 + kl : k0 + kl + 1],
                                tnh[:, hc, kl, :Qs],
                                wv_sb[:, hc : hc + 1],
                                start=(hc == 0),
                                stop=(hc == 3),
                            )
                            if hc == 0:
                                first = bi.ins
                            last = bi.ins
                        if prev_last is not None:
                            tile.add_dep_helper(
                                first, prev_last, sync=False,
                                reason="psc accumulation-group order",
                            )
                        prev_last = last
                    if g == min(1, ngroups - 1) and pending is not None:
                        epilogue(*pending)
                        pending = None
                pending = (s, psc)
            epilogue(*pending)

    nc.finalize()
    return nc


def kernel(queries, keys, values, valid_lens, Wq, Wk, wv):
    global LAST_RESULT
    queries = np.asarray(queries, dtype=np.float32)
    keys = np.asarray(keys, dtype=np.float32)
    values = np.asarray(values, dtype=np.float32)
    valid_lens = np.asarray(valid_lens, dtype=np.int32)
    Wq = np.asarray(Wq, dtype=np.float32)
    Wk = np.asarray(Wk, dtype=np.float32)
    wv = np.asarray(wv, dtype=np.float32)

    slots, content = _plan(valid_lens)
    S = len(slots)

    if slots not in _NC_CACHE:
        _NC_CACHE[slots] = _build_nc(slots)
    nc = _NC_CACHE[slots]

    # ---- host-side shard prep -------------------------------------------
    wq_bf = Wq.astype(BF16_NP)
    wk_bf = Wk.astype(BF16_NP)
    wv4 = np.ascontiguousarray(wv.reshape(4, 128).T).astype(BF16_NP)  # [128,4]

    in_maps = []
    for c in range(NCORES):
        qTm = np.zeros((S, D, 128), dtype=BF16_NP)
        kTm = np.zeros((S, D, 128), dtype=BF16_NP)
        valsm = np.zeros((S, K, D), dtype=BF16_NP)
        maskm = np.zeros((S, 128, K), dtype=np.float32)
        for s, (b, q0, qs, v) in enumerate(content[c]):
            qTm[s, :, :qs] = queries[b, q0 : q0 + qs].T.astype(BF16_NP)
            kTm[s, :, :v] = keys[b, :v].T.astype(BF16_NP)
            valsm[s, :v] = values[b, :v].astype(BF16_NP)
            maskm[s, :, v:] = NEG
        in_maps.append(
            {
                "qT": qTm,
                "kT": kTm,
                "vals": valsm,
                "wq": wq_bf,
                "wk": wk_bf,
                "wv4": wv4,
                "mask": maskm,
            }
        )

    res = run_bass_kernel_spmd(nc, in_maps, list(range(NCORES)))
    LAST_RESULT = res

    out = np.empty((B, Q, D), dtype=np.float32)
    for c in range(NCORES):
        o = np.asarray(res.results[c]["out"], dtype=np.float32)
        for s, (b, q0, qs, v) in enumerate(content[c]):
            out[b, q0 : q0 + qs] = o[s, :qs]
    return out


# revision 23
# speedup vs baseline: 1.1997x; 1.1369x over previous
"""Trainium2 Bass kernel for nn_AdditiveAttention (Bahdanau attention).

Reference computation (B=16, Q=128, K=128, D=512, H=512):
    q = queries @ Wq                     [B,Q,H]
    k = keys @ Wk                        [B,K,H]
    scores[b,q,k] = sum_h wv[h] * tanh(q[b,q,h] + k[b,k,h])
    attn = softmax over valid keys (k < valid_lens[b])
    out = attn @ values                  [B,Q,D]

Strategy (8 NeuronCores, SPMD data-parallel):
  Work is proportional to (#queries x valid_len) per batch, and queries are
  embarrassingly parallel (each query's softmax is independent).  Each batch
  is split into q-range fragments; fragments are sorted by valid_len and
  distributed over 8 cores x S uniform "slots" (one SPMD program).  Slot j
  has a fixed shape (Qs_j, V_j) = (fragment q-size, max valid_len in that
  slot across cores); shorter fragments are handled with an additive -1e9
  key mask.  Sorted assignment keeps slot V_j close to each member's
  valid_len, minimizing padded work.

  On-device per slot: project queriesT/keysT transposed ([h=partitions]),
  per key-column k: pre[h,q] = q_projT + k_col broadcast-add on DVE (grouped
  stride-0 tensor_tensor), tanh on ScalarE in big batched instructions, wv
  reduction on TensorE (lhsT = tanh tile [128h,Qs], rhs = wv chunk [128h,1]
  -> scores column, one independent matmul per (k, h-chunk) into 4
  per-chunk PSUM banks), bank-sum + mask on DVE, exp with fused accumulated
  sum on ScalarE, reciprocal on DVE, transpose of the exp matrix on
  TensorE, and a final attn @ values matmul scaled by 1/z.
  bf16 on PE/DVE with fp32 PSUM accumulation; tanh/exp are fp32 internally.
"""

import os
import sys
import types
import math
import numpy as np
import ml_dtypes

# ---------------------------------------------------------------------------
# axon NTFF profile hook (lets trace=True / BASS_TRACE=1 work in this image)
# ---------------------------------------------------------------------------
def _install_axon_hooks():
    if "antenv.axon_hooks" in sys.modules:
        return
    try:
        import trn_agent_boot.trn_boot as _tb

        _hooks = types.ModuleType("antenv.axon_hooks")
        _hook = _tb._ntff_profile_via_ctypes("/opt/axon/libaxon_pjrt.so")
        _hooks.get_axon_ntff_profile_hook = lambda: _hook
        _hooks.set_axon_ntff_profile_hook = lambda h: None
        sys.modules["antenv.axon_hooks"] = _hooks
    except Exception:
        pass


_install_axon_hooks()

import concourse.bass as bass
import concourse.bacc as bacc
import concourse.mybir as mybir
import concourse.tile as tile
import concourse.bass_utils as bass_utils
from concourse.bass_utils import run_bass_kernel_spmd
from concourse.masks import make_identity

# Avoid S3 artifact-upload attempts in the trace path.
bass_utils.upload_artifacts = lambda tmpdir: tmpdir

F32 = mybir.dt.float32
BF16 = mybir.dt.bfloat16
BF16_NP = ml_dtypes.bfloat16

B, Q, K, D, H = 16, 128, 128, 512, 512
NCORES = 8
KT = 32  # key-columns per tanh group
NEG = -1e9

_NC_CACHE: dict = {}
LAST_RESULT = None


def _plan(valid_lens):
    """Pick fragmentation scheme; return (slots, content).

    slots:   tuple of (Qs, V) uniform slot shapes
    content: per-core list of fragments (batch, q0, qs, v), one per slot
    """
    best = None
    for nsplit in (1, 2):
        S = (B * nsplit) // NCORES
        qs = Q // nsplit
        frags = [
            (b, i * qs, qs, int(valid_lens[b]))
            for b in range(B)
            for i in range(nsplit)
        ]
        frags.sort(key=lambda f: -f[3])
        slots = []
        content = [[] for _ in range(NCORES)]
        cost = 0.0
        for j in range(S):
            grp = frags[j * NCORES : (j + 1) * NCORES]
            V = max(f[3] for f in grp)
            slots.append((qs, V))
            cost += qs * V
            for c, f in enumerate(grp):
                content[c].append(f)
        cost += S * 600.0  # per-slot fixed overhead, in q*k units
        if best is None or cost < best[0]:
            best = (cost, tuple(slots), content)
    return best[1], best[2]


def _build_nc(slots):
    """Build + finalize the single-core SPMD program for the given slots."""
    S = len(slots)
    maxQs = max(q for q, _ in slots)
    nc = bacc.Bacc(None, target_bir_lowering=False, debug=False)

    qT = nc.declare_dram_parameter("qT", [S, D, 128], BF16, isOutput=False)
    kT = nc.declare_dram_parameter("kT", [S, D, 128], BF16, isOutput=False)
    vals = nc.declare_dram_parameter("vals", [S, K, D], BF16, isOutput=False)
    wq_d = nc.declare_dram_parameter("wq", [D, H], BF16, isOutput=False)
    wk_d = nc.declare_dram_parameter("wk", [D, H], BF16, isOutput=False)
    wv_d = nc.declare_dram_parameter("wv4", [128, 4], BF16, isOutput=False)
    mask_d = nc.declare_dram_parameter("mask", [S, 128, K], F32, isOutput=False)
    out_d = nc.declare_dram_parameter("out", [S, 128, D], F32, isOutput=True)

    Tanh = mybir.ActivationFunctionType.Tanh
    Exp = mybir.ActivationFunctionType.Exp
    stage_bufs = 3 if maxQs <= 64 else 2

    with tile.TileContext(nc) as tc:
        with (
            tc.tile_pool(name="const", bufs=1) as constp,
            tc.tile_pool(name="io", bufs=1) as iop,
            tc.tile_pool(name="proj", bufs=1) as projp,
            tc.tile_pool(name="stage", bufs=stage_bufs) as stagep,
            tc.tile_pool(name="sm", bufs=2) as smp,
            tc.tile_pool(name="ps_proj", bufs=2, space="PSUM") as ps_proj,
            tc.tile_pool(name="ps_sc", bufs=4, space="PSUM") as ps_sc,
            tc.tile_pool(name="ps_misc", bufs=1, space="PSUM") as ps_misc,
        ):
            # ---- constants & inputs -------------------------------------
            wq_sb = constp.tile([128, 4, H], BF16, tag="wq")
            nc.sync.dma_start(wq_sb[:], wq_d[:].rearrange("(c p) h -> p c h", p=128))
            wk_sb = constp.tile([128, 4, H], BF16, tag="wk")
            nc.sync.dma_start(wk_sb[:], wk_d[:].rearrange("(c p) h -> p c h", p=128))
            qt_sb = iop.tile([128, S, 4, 128], BF16, tag="qt")
            kt_sb = iop.tile([128, S, 4, 128], BF16, tag="kt")
            qT_r = qT[:].rearrange("s (c p) q -> p s c q", p=128)
            kT_r = kT[:].rearrange("s (c p) k -> p s c k", p=128)
            for s in range(S):
                nc.sync.dma_start(qt_sb[:, s], qT_r[:, s])
                nc.sync.dma_start(kt_sb[:, s], kT_r[:, s])
            wv_sb = constp.tile([128, 4], BF16, tag="wv")
            nc.sync.dma_start(wv_sb[:], wv_d[:])
            ident = constp.tile([128, 128], BF16, tag="ident")
            make_identity(nc, ident[:])
            vals_sb = iop.tile([128, S, D], BF16, tag="vals")
            nc.sync.dma_start(vals_sb[:], vals[:].rearrange("s k d -> k s d"))
            mask_sb = iop.tile([128, S, K], F32, tag="mask")
            nc.sync.dma_start(mask_sb[:], mask_d[:].rearrange("s p k -> p s k"))

            # ---- projections: projT[h,x] = sum_d W[d,h] * xT[d,x] -------
            # kproj2 holds each projected key DUPLICATED ([..., k, 2]) so
            # the broadcast-add can run in DVE 2x_1P packed mode: both
            # operands' innermost free dim is step-1 bf16 (in0 reads the
            # duplicated key pair, in1 reads adjacent query pairs), while
            # the pre/tanh tiles stay fully contiguous per key column.
            qproj = projp.tile([128, S, 4, 128], BF16, tag="qproj")
            kproj2 = projp.tile([128, S, 4, 128, 2], BF16, tag="kproj")
            nc.vector.memset(kproj2[:], 0.0)
            for s in range(S):
                Qs, V = slots[s]
                for hc in range(4):
                    pq = ps_proj.tile([128, 128], F32, tag="pp")
                    for dc in range(4):
                        nc.tensor.matmul(
                            pq[:, :Qs],
                            wq_sb[:, dc, hc * 128 : (hc + 1) * 128],
                            qt_sb[:, s, dc, :Qs],
                            start=(dc == 0),
                            stop=(dc == 3),
                        )
                    nc.scalar.copy(qproj[:, s, hc, :Qs], pq[:, :Qs])
                    pk = ps_proj.tile([128, 128], F32, tag="pp")
                    for dc in range(4):
                        nc.tensor.matmul(
                            pk[:, :V],
                            wk_sb[:, dc, hc * 128 : (hc + 1) * 128],
                            kt_sb[:, s, dc, :V],
                            start=(dc == 0),
                            stop=(dc == 3),
                        )
                    nc.scalar.copy(
                        kproj2[:, s, hc, :V, :],
                        pk[:, :V].unsqueeze(2).broadcast_to((128, V, 2)),
                    )

            # persistent softmax state
            e_sb = projp.tile([128, S, 128], BF16, tag="e")
            nc.vector.memset(e_sb[:], 0.0)
            rz = projp.tile([128, S], F32, tag="rz")

            # ---- main loop ----------------------------------------------
            # Slot epilogues (softmax + output) are emitted one slot late:
            # engines are in-order, so emitting an epilogue (which waits on
            # the slot's full PE matmul tail) before the next slot's group
            # work would head-of-line-block every engine at each slot
            # boundary.
            def epilogue(s, psc):
                Qs, V = slots[s]
                msc = smp.tile([128, K], F32, tag="msc", name=f"msc{s}")
                nc.vector.tensor_add(
                    msc[:Qs, :V], psc[:Qs, :V], mask_sb[:Qs, s, :V]
                )
                z = smp.tile([128, 1], F32, tag="z", name=f"z{s}")
                nc.scalar.activation(
                    e_sb[:Qs, s, :V], msc[:Qs, :V], Exp, accum_out=z[:Qs, :]
                )
                nc.vector.reciprocal(rz[:Qs, s : s + 1], z[:Qs, :])
                pt = ps_misc.tile([128, 128], BF16, tag="pt", name=f"pt{s}")
                nc.tensor.transpose(pt[:], e_sb[:, s, :], ident[:])
                eT = smp.tile([128, 128], BF16, tag="eT", name=f"eT{s}")
                nc.vector.tensor_copy(eT[:], pt[:])
                po = ps_misc.tile([128, D], F32, tag="po", name=f"po{s}")
                nc.tensor.matmul(
                    po[:, :], eT[:V, :], vals_sb[:V, s, :], start=True, stop=True
                )
                o_sb = smp.tile([128, D], F32, tag="o", name=f"o{s}")
                nc.vector.tensor_scalar_mul(
                    o_sb[:Qs, :], po[:Qs, :], rz[:Qs, s : s + 1]
                )
                nc.sync.dma_start(out_d[s, :Qs], o_sb[:Qs, :])

            pending = None  # (s, psc) awaiting epilogue
            for s in range(S):
                Qs, V = slots[s]
                ngroups = math.ceil(V / KT)
                # single score bank per slot: per key, 4 h-chunk matmuls
                # accumulate into one column; PE issue order pinned so a
                # start=True (whole-bank has_written clear) never lands
                # inside another key's accumulation group.
                psc = ps_sc.tile([128, K], F32, tag="psc")
                prev_last = None
                for g in range(ngroups):
                    k0 = g * KT
                    Kg = min(KT, V - k0)
                    nflat = Kg * Qs
                    pre = stagep.tile([128, 4, KT * maxQs], BF16, tag="pre")
                    tnh = stagep.tile([128, 4, KT * maxQs], BF16, tag="tnh")
                    for hc in range(4):
                        # pre[h, kl, qp, j] = kproj[h, k0+kl] + qproj[h, 2qp+j]
                        in0 = (
                            kproj2[:, s, hc, k0 : k0 + Kg, :]
                            .unsqueeze(2)
                            .broadcast_to((128, Kg, Qs // 2, 2))
                        )
                        in1 = (
                            qproj[:, s, hc, :Qs]
                            .rearrange("p (qp j) -> p qp j", j=2)
                            .unsqueeze(1)
                            .broadcast_to((128, Kg, Qs // 2, 2))
                        )
                        out = pre[:, hc, :nflat].rearrange(
                            "p (kl qp j) -> p kl qp j", qp=Qs // 2, j=2
                        )
                        nc.vector.tensor_add(out, in0, in1)
                    nc.scalar.activation(
                        tnh[:, 0:2, :nflat], pre[:, 0:2, :nflat], Tanh
                    )
                    nc.scalar.activation(
                        tnh[:, 2:4, :nflat], pre[:, 2:4, :nflat], Tanh
                    )
                    tnh3 = tnh[:, :, :nflat].rearrange(
                        "p hc (kl q) -> p hc kl q", q=Qs
                    )
                    for kl in range(Kg):
                        first = None
                        for hc in range(4):
                            bi = nc.tensor.matmul(
                                psc[:Qs, k0 + kl : k0 + kl + 1],
                                tnh3[:, hc, kl, :],
                                wv_sb[:, hc : hc + 1],
                                start=(hc == 0),
                                stop=(hc == 3),
                            )
                            if hc == 0:
                                first = bi.ins
                            last = bi.ins
                        if prev_last is not None:
                            tile.add_dep_helper(
                                first, prev_last, sync=False,
                                reason="psc accumulation-group order",
                            )
                        prev_last = last
                    if g == min(1, ngroups - 1) and pending is not None:
                        epilogue(*pending)
                        pending = None
                pending = (s, psc)
            epilogue(*pending)

    nc.finalize()
    return nc


def kernel(queries, keys, values, valid_lens, Wq, Wk, wv):
    global LAST_RESULT
    queries = np.asarray(queries, dtype=np.float32)
    keys = np.asarray(keys, dtype=np.float32)
    values = np.asarray(values, dtype=np.float32)
    valid_lens = np.asarray(valid_lens, dtype=np.int32)
    Wq = np.asarray(Wq, dtype=np.float32)
    Wk = np.asarray(Wk, dtype=np.float32)
    wv = np.asarray(wv, dtype=np.float32)

    slots, content = _plan(valid_lens)
    S = len(slots)

    if slots not in _NC_CACHE:
        _NC_CACHE[slots] = _build_nc(slots)
    nc = _NC_CACHE[slots]

    # ---- host-side shard prep -------------------------------------------
    wq_bf = Wq.astype(BF16_NP)
    wk_bf = Wk.astype(BF16_NP)
    wv4 = np.ascontiguousarray(wv.reshape(4, 128).T).astype(BF16_NP)  # [128,4]

    in_maps = []
    for c in range(NCORES):
        qTm = np.zeros((S, D, 128), dtype=BF16_NP)
        kTm = np.zeros((S, D, 128), dtype=BF16_NP)
        valsm = np.zeros((S, K, D), dtype=BF16_NP)
        maskm = np.zeros((S, 128, K), dtype=np.float32)
        for s, (b, q0, qs, v) in enumerate(content[c]):
            qTm[s, :, :qs] = queries[b, q0 : q0 + qs].T.astype(BF16_NP)
            kTm[s, :, :v] = keys[b, :v].T.astype(BF16_NP)
            valsm[s, :v] = values[b, :v].astype(BF16_NP)
            maskm[s, :, v:] = NEG
        in_maps.append(
            {
                "qT": qTm,
                "kT": kTm,
                "vals": valsm,
                "wq": wq_bf,
                "wk": wk_bf,
                "wv4": wv4,
                "mask": maskm,
            }
        )

    res = run_bass_kernel_spmd(nc, in_maps, list(range(NCORES)))
    LAST_RESULT = res

    out = np.empty((B, Q, D), dtype=np.float32)
    for c in range(NCORES):
        o = np.asarray(res.results[c]["out"], dtype=np.float32)
        for s, (b, q0, qs, v) in enumerate(content[c]):
            out[b, q0 : q0 + qs] = o[s, :qs]
    return out


# revision 25
# speedup vs baseline: 1.3145x; 1.0957x over previous
"""Trainium2 Bass kernel for nn_AdditiveAttention (Bahdanau attention).

Reference computation (B=16, Q=128, K=128, D=512, H=512):
    q = queries @ Wq                     [B,Q,H]
    k = keys @ Wk                        [B,K,H]
    scores[b,q,k] = sum_h wv[h] * tanh(q[b,q,h] + k[b,k,h])
    attn = softmax over valid keys (k < valid_lens[b])
    out = attn @ values                  [B,Q,D]

Strategy (8 NeuronCores, SPMD, key-split data parallelism):
  Work per batch is proportional to its valid_len, and softmax over keys
  decomposes into per-key-range partials (no max subtraction is needed:
  |scores| <= sum|wv| is small).  Each batch's valid key range is split
  into contiguous fragments; fragments are packed into 8 cores x S
  uniform "slots" (cells), one fragment per cell, one SPMD program.  A
  cell computes the UNNORMALIZED partial o = exp(scores) @ values and
  z = sum(exp(scores)) over its key range; the host combines
  out[b] = sum_frag(o) / sum_frag(z).  Slot j has fixed key capacity V_j
  (a host-side search minimizes sum V_j); shorter fragments are masked
  with an additive -1e9.

  On-device per slot: project queriesT/keysT transposed ([h=partitions]);
  per key column k: pre[h,q] = q_projT + k_col broadcast-add on DVE in
  2x_1P packed mode (kproj stored as duplicated (k,k) pairs; qproj read
  as adjacent (q,q+1) pairs -> both operands innermost step-1 bf16);
  tanh on ScalarE in big batched instructions; wv reduction on TensorE
  (lhsT = tanh tile [128h,128q], rhs = wv chunk [128h,1] -> one PSUM
  score column per (k, h-chunk), accumulation order pinned); masked exp
  with fused accumulated sum on ScalarE; transpose of the exp matrix on
  TensorE; and the final exp @ values matmul.
  bf16 on PE/DVE with fp32 PSUM accumulation; tanh/exp fp32 internally.
"""

import os
import sys
import types
import math
import bisect
import numpy as np
import ml_dtypes

# ---------------------------------------------------------------------------
# axon NTFF profile hook (lets trace=True / BASS_TRACE=1 work in this image)
# ---------------------------------------------------------------------------
def _install_axon_hooks():
    if "antenv.axon_hooks" in sys.modules:
        return
    try:
        import trn_agent_boot.trn_boot as _tb

        _hooks = types.ModuleType("antenv.axon_hooks")
        _hook = _tb._ntff_profile_via_ctypes("/opt/axon/libaxon_pjrt.so")
        _hooks.get_axon_ntff_profile_hook = lambda: _hook
        _hooks.set_axon_ntff_profile_hook = lambda h: None
        sys.modules["antenv.axon_hooks"] = _hooks
    except Exception:
        pass


_install_axon_hooks()

import concourse.bass as bass
import concourse.bacc as bacc
import concourse.mybir as mybir
import concourse.tile as tile
import concourse.bass_utils as bass_utils
from concourse.bass_utils import run_bass_kernel_spmd
from concourse.masks import make_identity

# Avoid S3 artifact-upload attempts in the trace path.
bass_utils.upload_artifacts = lambda tmpdir: tmpdir

F32 = mybir.dt.float32
BF16 = mybir.dt.bfloat16
BF16_NP = ml_dtypes.bfloat16

B, Q, K, D, H = 16, 128, 128, 512, 512
NCORES = 8
KT = 16  # key-columns per tanh group
NEG = -1e9

_NC_CACHE: dict = {}
LAST_RESULT = None


def _pack(vl, caps):
    """Pack each batch's valid keys as contiguous ranges into cells (one
    range per cell).  Best-fit: smallest cell that fits the remainder,
    else the largest cell.  Returns content[core][slot] = (b, k0, klen)
    (b = -1 for empty cells) or None if infeasible."""
    cells = []
    for j, cap in enumerate(caps):
        for c in range(NCORES):
            cells.append((cap, c, j))
    avail = sorted(cells)
    content = [[(-1, 0, 0)] * len(caps) for _ in range(NCORES)]
    for b in np.argsort(-vl, kind="stable"):
        rem = int(vl[b])
        k0 = 0
        while rem > 0:
            if not avail:
                return None
            caps_list = [x[0] for x in avail]
            i = bisect.bisect_left(caps_list, rem)
            if i < len(avail):
                cap, c, j = avail.pop(i)
                take = rem
            else:
                cap, c, j = avail.pop()
                take = cap
            content[c][j] = (int(b), k0, take)
            k0 += take
            rem -= take
    return content


def _plan(valid_lens):
    """Search slot capacities minimizing padded work; returns
    (slots, content) with slots = tuple of V_j."""
    vl = np.asarray(valid_lens)
    cand = set()
    for v in vl:
        for k in (1, 2, 3, 4):
            cand.add(int(math.ceil(int(v) / k)))
    cand = sorted(x for x in cand if x >= 1)
    import itertools

    tot = int(vl.sum())
    best = None
    for S in (2, 3, 4):
        for caps in itertools.combinations_with_replacement(
            sorted(cand, reverse=True), S
        ):
            sv = sum(caps)
            if NCORES * sv < tot:
                continue
            if best is not None and Q * sv + S * 700.0 >= best[0]:
                continue
            content = _pack(vl, caps)
            if content is None:
                continue
            best = (Q * sv + S * 700.0, caps, content)
    return best[1], best[2]


def _build_nc(caps):
    """Build + finalize the single-core SPMD program for slot caps."""
    S = len(caps)
    nc = bacc.Bacc(None, target_bir_lowering=False, debug=False)

    qT = nc.declare_dram_parameter("qT", [S, D, Q], BF16, isOutput=False)
    kT = nc.declare_dram_parameter("kT", [S, D, K], BF16, isOutput=False)
    vals = nc.declare_dram_parameter("vals", [S, K, D], BF16, isOutput=False)
    wq_d = nc.declare_dram_parameter("wq", [D, H], BF16, isOutput=False)
    wk_d = nc.declare_dram_parameter("wk", [D, H], BF16, isOutput=False)
    wv_d = nc.declare_dram_parameter("wv4", [128, 4], BF16, isOutput=False)
    mask_d = nc.declare_dram_parameter("mask", [S, 128, K], F32, isOutput=False)
    out_d = nc.declare_dram_parameter("out", [S, Q, D], F32, isOutput=True)
    outz_d = nc.declare_dram_parameter("outz", [S, Q, 1], F32, isOutput=True)

    Tanh = mybir.ActivationFunctionType.Tanh
    Exp = mybir.ActivationFunctionType.Exp

    with tile.TileContext(nc) as tc:
        with (
            tc.tile_pool(name="const", bufs=1) as constp,
            tc.tile_pool(name="io", bufs=1) as iop,
            tc.tile_pool(name="proj", bufs=1) as projp,
            tc.tile_pool(name="stage", bufs=3) as stagep,
            tc.tile_pool(name="sm", bufs=2) as smp,
            tc.tile_pool(name="ps_proj", bufs=2, space="PSUM") as ps_proj,
            tc.tile_pool(name="ps_sc", bufs=4, space="PSUM") as ps_sc,
            tc.tile_pool(name="ps_misc", bufs=1, space="PSUM") as ps_misc,
        ):
            # ---- constants & inputs -------------------------------------
            wq_sb = constp.tile([128, 4, H], BF16, tag="wq")
            nc.sync.dma_start(wq_sb[:], wq_d[:].rearrange("(c p) h -> p c h", p=128))
            wk_sb = constp.tile([128, 4, H], BF16, tag="wk")
            nc.sync.dma_start(wk_sb[:], wk_d[:].rearrange("(c p) h -> p c h", p=128))
            qt_sb = iop.tile([128, S, 4, Q], BF16, tag="qt")
            kt_sb = iop.tile([128, S, 4, K], BF16, tag="kt")
            qT_r = qT[:].rearrange("s (c p) q -> p s c q", p=128)
            kT_r = kT[:].rearrange("s (c p) k -> p s c k", p=128)
            for s in range(S):
                nc.sync.dma_start(qt_sb[:, s], qT_r[:, s])
                nc.sync.dma_start(kt_sb[:, s], kT_r[:, s])
            wv_sb = constp.tile([128, 4], BF16, tag="wv")
            nc.sync.dma_start(wv_sb[:], wv_d[:])
            ident = constp.tile([128, 128], BF16, tag="ident")
            make_identity(nc, ident[:])
            vals_sb = iop.tile([128, S, D], BF16, tag="vals")
            nc.sync.dma_start(vals_sb[:], vals[:].rearrange("s k d -> k s d"))
            mask_sb = iop.tile([128, S, K], F32, tag="mask")
            nc.sync.dma_start(mask_sb[:], mask_d[:].rearrange("s p k -> p s k"))

            # ---- projections: projT[h,x] = sum_d W[d,h] * xT[d,x] -------
            # kproj2 holds each projected key DUPLICATED ([..., k, 2]) so
            # the broadcast-add runs in DVE 2x_1P packed mode: in0 reads
            # the duplicated key pair, in1 adjacent query pairs, keeping
            # pre/tanh tiles contiguous per key column.
            qproj = projp.tile([128, S, 4, Q], BF16, tag="qproj")
            kproj2 = projp.tile([128, S, 4, K, 2], BF16, tag="kproj")
            nc.vector.memset(kproj2[:], 0.0)
            for s in range(S):
                V = caps[s]
                for hc in range(4):
                    pq = ps_proj.tile([128, 128], F32, tag="pp")
                    for dc in range(4):
                        nc.tensor.matmul(
                            pq[:],
                            wq_sb[:, dc, hc * 128 : (hc + 1) * 128],
                            qt_sb[:, s, dc, :],
                            start=(dc == 0),
                            stop=(dc == 3),
                        )
                    nc.vector.tensor_copy(qproj[:, s, hc, :], pq[:])
                    pk = ps_proj.tile([128, 128], F32, tag="pp")
                    for dc in range(4):
                        nc.tensor.matmul(
                            pk[:, :V],
                            wk_sb[:, dc, hc * 128 : (hc + 1) * 128],
                            kt_sb[:, s, dc, :V],
                            start=(dc == 0),
                            stop=(dc == 3),
                        )
                    nc.vector.tensor_copy(
                        kproj2[:, s, hc, :V, :],
                        pk[:, :V].unsqueeze(2).broadcast_to((128, V, 2)),
                    )

            # persistent softmax state
            e_sb = projp.tile([128, S, K], BF16, tag="e")
            nc.vector.memset(e_sb[:], 0.0)

            # ---- main loop ----------------------------------------------
            # Slot epilogues are emitted one slot late: engines are
            # in-order, so emitting an epilogue (which waits on the slot's
            # full PE matmul tail) before the next slot's group work would
            # head-of-line-block every engine at the slot boundary.
            def epilogue(s, psc):
                V = caps[s]
                msc = smp.tile([128, K], F32, tag="msc", name=f"msc{s}")
                nc.vector.tensor_add(
                    msc[:, :V], psc[:, :V], mask_sb[:, s, :V]
                )
                z = smp.tile([128, 1], F32, tag="z", name=f"z{s}")
                nc.scalar.activation(
                    e_sb[:, s, :V], msc[:, :V], Exp, accum_out=z[:]
                )
                pt = ps_misc.tile([128, 128], BF16, tag="pt", name=f"pt{s}")
                nc.tensor.transpose(pt[:], e_sb[:, s, :], ident[:])
                eT = smp.tile([128, 128], BF16, tag="eT", name=f"eT{s}")
                nc.vector.tensor_copy(eT[:], pt[:])
                po = ps_misc.tile([128, D], F32, tag="po", name=f"po{s}")
                nc.tensor.matmul(
                    po[:, :], eT[:V, :], vals_sb[:V, s, :], start=True, stop=True
                )
                o_sb = smp.tile([128, D], F32, tag="o", name=f"o{s}")
                nc.vector.tensor_copy(o_sb[:], po[:])
                nc.sync.dma_start(out_d[s], o_sb[:])
                nc.sync.dma_start(outz_d[s], z[:])

            pending = None
            for s in range(S):
                V = caps[s]
                ngroups = math.ceil(V / KT)
                psc = ps_sc.tile([128, K], F32, tag="psc", name=f"psc{s}")
                prev_last = None
                for g in range(ngroups):
                    k0 = g * KT
                    Kg = min(KT, V - k0)
                    nflat = Kg * Q
                    pre = stagep.tile([128, 4, KT * Q], BF16, tag="pre")
                    tnh = stagep.tile([128, 4, KT * Q], BF16, tag="tnh")
                    for hc in range(4):
                        # pre[h, kl, qp, j] = kproj[h, k0+kl] + qproj[h, 2qp+j]
                        in0 = (
                            kproj2[:, s, hc, k0 : k0 + Kg, :]
                            .unsqueeze(2)
                            .broadcast_to((128, Kg, Q // 2, 2))
                        )
                        in1 = (
                            qproj[:, s, hc, :]
                            .rearrange("p (qp j) -> p qp j", j=2)
                            .unsqueeze(1)
                            .broadcast_to((128, Kg, Q // 2, 2))
                        )
                        out = pre[:, hc, :nflat].rearrange(
                            "p (kl qp j) -> p kl qp j", qp=Q // 2, j=2
                        )
                        nc.vector.tensor_add(out, in0, in1)
                    nc.scalar.activation(
                        tnh[:, 0:2, :nflat], pre[:, 0:2, :nflat], Tanh
                    )
                    nc.scalar.activation(
                        tnh[:, 2:4, :nflat], pre[:, 2:4, :nflat], Tanh
                    )
                    tnh3 = tnh[:, :, :nflat].rearrange(
                        "p hc (kl q) -> p hc kl q", q=Q
                    )
                    for kl in range(Kg):
                        first = None
                        for hc in range(4):
                            bi = nc.tensor.matmul(
                                psc[:, k0 + kl : k0 + kl + 1],
                                tnh3[:, hc, kl, :],
                                wv_sb[:, hc : hc + 1],
                                start=(hc == 0),
                                stop=(hc == 3),
                            )
                            if hc == 0:
                                first = bi.ins
                            last = bi.ins
                        if prev_last is not None:
                            tile.add_dep_helper(
                                first, prev_last, sync=False,
                                reason="psc accumulation-group order",
                            )
                        prev_last = last
                if pending is not None:
                    epilogue(*pending)
                pending = (s, psc)
            epilogue(*pending)

    nc.finalize()
    return nc


def kernel(queries, keys, values, valid_lens, Wq, Wk, wv):
    global LAST_RESULT
    queries = np.asarray(queries, dtype=np.float32)
    keys = np.asarray(keys, dtype=np.float32)
    values = np.asarray(values, dtype=np.float32)
    valid_lens = np.asarray(valid_lens, dtype=np.int32)
    Wq = np.asarray(Wq, dtype=np.float32)
    Wk = np.asarray(Wk, dtype=np.float32)
    wv = np.asarray(wv, dtype=np.float32)

    caps, content = _plan(valid_lens)
    S = len(caps)

    if caps not in _NC_CACHE:
        _NC_CACHE[caps] = _build_nc(caps)
    nc = _NC_CACHE[caps]

    # ---- host-side shard prep -------------------------------------------
    wq_bf = Wq.astype(BF16_NP)
    wk_bf = Wk.astype(BF16_NP)
    wv4 = np.ascontiguousarray(wv.reshape(4, 128).T).astype(BF16_NP)  # [128,4]
    qTt = {
        b: np.ascontiguousarray(queries[b].T).astype(BF16_NP) for b in range(B)
    }

    in_maps = []
    for c in range(NCORES):
        qTm = np.zeros((S, D, Q), dtype=BF16_NP)
        kTm = np.zeros((S, D, K), dtype=BF16_NP)
        valsm = np.zeros((S, K, D), dtype=BF16_NP)
        maskm = np.zeros((S, 128, K), dtype=np.float32)
        for s, (b, k0, klen) in enumerate(content[c]):
            if b < 0:
                maskm[s, :, :] = NEG
                continue
            qTm[s] = qTt[b]
            kTm[s, :, :klen] = keys[b, k0 : k0 + klen].T.astype(BF16_NP)
            valsm[s, :klen] = values[b, k0 : k0 + klen].astype(BF16_NP)
            maskm[s, :, klen:] = NEG
        in_maps.append(
            {
                "qT": qTm,
                "kT": kTm,
                "vals": valsm,
                "wq": wq_bf,
                "wk": wk_bf,
                "wv4": wv4,
                "mask": maskm,
            }
        )

    res = run_bass_kernel_spmd(nc, in_maps, list(range(NCORES)))
    LAST_RESULT = res

    O = np.zeros((B, Q, D), dtype=np.float64)
    Z = np.zeros((B, Q, 1), dtype=np.float64)
    for c in range(NCORES):
        o = np.asarray(res.results[c]["out"], dtype=np.float64)
        z = np.asarray(res.results[c]["outz"], dtype=np.float64)
        for s, (b, k0, klen) in enumerate(content[c]):
            if b < 0:
                continue
            O[b] += o[s]
            Z[b] += z[s]
    return (O / Z).astype(np.float32)


# revision 28
# speedup vs baseline: 1.4966x; 1.1386x over previous
"""Trainium2 Bass kernel for nn_AdditiveAttention (Bahdanau attention).

Reference computation (B=16, Q=128, K=128, D=512, H=512):
    q = queries @ Wq                     [B,Q,H]
    k = keys @ Wk                        [B,K,H]
    scores[b,q,k] = sum_h wv[h] * tanh(q[b,q,h] + k[b,k,h])
    attn = softmax over valid keys (k < valid_lens[b])
    out = attn @ values                  [B,Q,D]

Strategy (8 NeuronCores, SPMD, key-split data parallelism):
  Work per batch is proportional to its valid_len, and softmax over keys
  decomposes into per-key-range partials (no max subtraction is needed:
  |scores| <= sum|wv| is small).  Each batch's valid key range is split
  into contiguous fragments; fragments are packed into 8 cores x S
  uniform "slots" (cells), one fragment per cell, one SPMD program.  A
  cell computes the UNNORMALIZED partial o = exp(scores) @ values and
  z = sum(exp(scores)) over its key range; the host combines
  out[b] = sum_frag(o) / sum_frag(z).  Slot j has fixed key capacity V_j
  (a host-side search minimizes sum V_j); shorter fragments are masked
  with an additive -1e9.

  On-device per slot: project queriesT/keysT transposed ([h=partitions]);
  per key column k: pre[h,q] = q_projT + k_col broadcast-add on DVE in
  2x_1P packed mode (kproj stored as duplicated (k,k) pairs; qproj read
  as adjacent (q,q+1) pairs -> both operands innermost step-1 bf16);
  tanh on ScalarE in big batched instructions; wv reduction on TensorE
  (lhsT = tanh tile [128h,128q], rhs = wv chunk [128h,1] -> one PSUM
  score column per (k, h-chunk), accumulation order pinned); masked exp
  with fused accumulated sum on ScalarE; transpose of the exp matrix on
  TensorE; and the final exp @ values matmul.
  bf16 on PE/DVE with fp32 PSUM accumulation; tanh/exp fp32 internally.
"""

import os
import sys
import types
import math
import bisect
import numpy as np
import ml_dtypes

# ---------------------------------------------------------------------------
# axon NTFF profile hook (lets trace=True / BASS_TRACE=1 work in this image)
# ---------------------------------------------------------------------------
def _install_axon_hooks():
    if "antenv.axon_hooks" in sys.modules:
        return
    try:
        import trn_agent_boot.trn_boot as _tb

        _hooks = types.ModuleType("antenv.axon_hooks")
        _hook = _tb._ntff_profile_via_ctypes("/opt/axon/libaxon_pjrt.so")
        _hooks.get_axon_ntff_profile_hook = lambda: _hook
        _hooks.set_axon_ntff_profile_hook = lambda h: None
        sys.modules["antenv.axon_hooks"] = _hooks
    except Exception:
        pass


_install_axon_hooks()

import concourse.bass as bass
import concourse.bacc as bacc
import concourse.mybir as mybir
import concourse.tile as tile
import concourse.bass_utils as bass_utils
from concourse.bass_utils import run_bass_kernel_spmd
from concourse.masks import make_identity

# Avoid S3 artifact-upload attempts in the trace path.
bass_utils.upload_artifacts = lambda tmpdir: tmpdir

F32 = mybir.dt.float32
BF16 = mybir.dt.bfloat16
BF16_NP = ml_dtypes.bfloat16

B, Q, K, D, H = 16, 128, 128, 512, 512
NCORES = 8
KT = 16  # key-columns per tanh group
NEG = -1e9

_NC_CACHE: dict = {}
LAST_RESULT = None


def _pack(vl, caps):
    """Pack each batch's valid keys as contiguous ranges into cells (one
    range per cell).  Best-fit: smallest cell that fits the remainder,
    else the largest cell.  Returns content[core][slot] = (b, k0, klen)
    (b = -1 for empty cells) or None if infeasible."""
    cells = []
    for j, cap in enumerate(caps):
        for c in range(NCORES):
            cells.append((cap, c, j))
    avail = sorted(cells)
    content = [[(-1, 0, 0)] * len(caps) for _ in range(NCORES)]
    for b in np.argsort(-vl, kind="stable"):
        rem = int(vl[b])
        k0 = 0
        while rem > 0:
            if not avail:
                return None
            caps_list = [x[0] for x in avail]
            i = bisect.bisect_left(caps_list, rem)
            if i < len(avail):
                cap, c, j = avail.pop(i)
                take = rem
            else:
                cap, c, j = avail.pop()
                take = cap
            content[c][j] = (int(b), k0, take)
            k0 += take
            rem -= take
    return content


def _plan(valid_lens):
    """Search slot capacities minimizing padded work; returns
    (slots, content) with slots = tuple of V_j."""
    vl = np.asarray(valid_lens)
    cand = set()
    for v in vl:
        for k in (1, 2, 3, 4):
            cand.add(int(math.ceil(int(v) / k)))
    cand = sorted(x for x in cand if x >= 1)
    import itertools

    tot = int(vl.sum())
    best = None
    for S in (2, 3, 4):
        for caps in itertools.combinations_with_replacement(
            sorted(cand, reverse=True), S
        ):
            sv = sum(caps)
            if NCORES * sv < tot:
                continue
            if best is not None and Q * sv + S * 700.0 >= best[0]:
                continue
            content = _pack(vl, caps)
            if content is None:
                continue
            best = (Q * sv + S * 700.0, caps, content)
    return best[1], best[2]


def _build_nc(caps):
    """Build + finalize the single-core SPMD program for slot caps."""
    S = len(caps)
    nc = bacc.Bacc(None, target_bir_lowering=False, debug=False)

    qT = nc.declare_dram_parameter("qT", [S, D, Q], BF16, isOutput=False)
    kT = nc.declare_dram_parameter("kT", [S, D, K], BF16, isOutput=False)
    vals = nc.declare_dram_parameter("vals", [S, K, D], BF16, isOutput=False)
    wq_d = nc.declare_dram_parameter("wq", [D, H], BF16, isOutput=False)
    wk_d = nc.declare_dram_parameter("wk", [D, H], BF16, isOutput=False)
    wv_d = nc.declare_dram_parameter("wv4", [128, 4], BF16, isOutput=False)
    mask_d = nc.declare_dram_parameter("mask", [S, 128, K], F32, isOutput=False)
    out_d = nc.declare_dram_parameter("out", [S, Q, D], F32, isOutput=True)
    outz_d = nc.declare_dram_parameter("outz", [S, Q, 1], F32, isOutput=True)

    Tanh = mybir.ActivationFunctionType.Tanh
    Exp = mybir.ActivationFunctionType.Exp

    with tile.TileContext(nc) as tc:
        with (
            tc.tile_pool(name="const", bufs=1) as constp,
            tc.tile_pool(name="io", bufs=1) as iop,
            tc.tile_pool(name="proj", bufs=1) as projp,
            tc.tile_pool(name="stage", bufs=3) as stagep,
            tc.tile_pool(name="sm", bufs=2) as smp,
            tc.tile_pool(name="ps_proj", bufs=2, space="PSUM") as ps_proj,
            tc.tile_pool(name="ps_sc", bufs=4, space="PSUM") as ps_sc,
            tc.tile_pool(name="ps_misc", bufs=1, space="PSUM") as ps_misc,
        ):
            # ---- constants & inputs -------------------------------------
            wq_sb = constp.tile([128, 4, H], BF16, tag="wq")
            nc.sync.dma_start(wq_sb[:], wq_d[:].rearrange("(c p) h -> p c h", p=128))
            wk_sb = constp.tile([128, 4, H], BF16, tag="wk")
            nc.sync.dma_start(wk_sb[:], wk_d[:].rearrange("(c p) h -> p c h", p=128))
            qt_sb = iop.tile([128, S, 4, Q], BF16, tag="qt")
            kt_sb = iop.tile([128, S, 4, K], BF16, tag="kt")
            qT_r = qT[:].rearrange("s (c p) q -> p s c q", p=128)
            kT_r = kT[:].rearrange("s (c p) k -> p s c k", p=128)
            for s in range(S):
                nc.sync.dma_start(qt_sb[:, s], qT_r[:, s])
                nc.sync.dma_start(kt_sb[:, s], kT_r[:, s])
            wv_sb = constp.tile([128, 4], BF16, tag="wv")
            nc.sync.dma_start(wv_sb[:], wv_d[:])
            ident = constp.tile([128, 128], BF16, tag="ident")
            make_identity(nc, ident[:])
            vals_sb = iop.tile([128, S, D], BF16, tag="vals")
            nc.sync.dma_start(vals_sb[:], vals[:].rearrange("s k d -> k s d"))
            mask_sb = iop.tile([128, S, K], F32, tag="mask")
            nc.sync.dma_start(mask_sb[:], mask_d[:].rearrange("s p k -> p s k"))

            # ---- projections: projT[h,x] = sum_d W[d,h] * xT[d,x] -------
            # kproj2 holds each projected key DUPLICATED ([..., k, 2]) so
            # the broadcast-add runs in DVE 2x_1P packed mode: in0 reads
            # the duplicated key pair, in1 adjacent query pairs, keeping
            # pre/tanh tiles contiguous per key column.
            qproj = projp.tile([128, S, 4, Q], BF16, tag="qproj")
            kproj2 = projp.tile([128, S, 4, K, 2], BF16, tag="kproj")
            for s in range(S):
                V = caps[s]
                for hc in range(4):
                    pq = ps_proj.tile([128, 128], F32, tag="pp")
                    for dc in range(4):
                        nc.tensor.matmul(
                            pq[:],
                            wq_sb[:, dc, hc * 128 : (hc + 1) * 128],
                            qt_sb[:, s, dc, :],
                            start=(dc == 0),
                            stop=(dc == 3),
                        )
                    nc.vector.tensor_copy(qproj[:, s, hc, :], pq[:])
                    pk = ps_proj.tile([128, 128], F32, tag="pp")
                    for dc in range(4):
                        nc.tensor.matmul(
                            pk[:, :V],
                            wk_sb[:, dc, hc * 128 : (hc + 1) * 128],
                            kt_sb[:, s, dc, :V],
                            start=(dc == 0),
                            stop=(dc == 3),
                        )
                    nc.vector.tensor_copy(
                        kproj2[:, s, hc, :V, :],
                        pk[:, :V].unsqueeze(2).broadcast_to((128, V, 2)),
                    )

            # persistent softmax state (cols >= V are never read into live
            # results: the output matmul contracts over eT[:V] only)
            e_sb = projp.tile([128, S, K], BF16, tag="e")

            # ---- main loop ----------------------------------------------
            # Slot epilogues are emitted one slot late: engines are
            # in-order, so emitting an epilogue (which waits on the slot's
            # full PE matmul tail) before the next slot's group work would
            # head-of-line-block every engine at the slot boundary.
            def epilogue(s, psc):
                V = caps[s]
                msc = smp.tile([128, K], F32, tag="msc", name=f"msc{s}")
                nc.vector.tensor_add(
                    msc[:, :V], psc[:, :V], mask_sb[:, s, :V]
                )
                z = smp.tile([128, 1], F32, tag="z", name=f"z{s}")
                nc.scalar.activation(
                    e_sb[:, s, :V], msc[:, :V], Exp, accum_out=z[:]
                )
                pt = ps_misc.tile([128, 128], BF16, tag="pt", name=f"pt{s}")
                nc.tensor.transpose(pt[:], e_sb[:, s, :], ident[:])
                eT = smp.tile([128, 128], BF16, tag="eT", name=f"eT{s}")
                nc.vector.tensor_copy(eT[:], pt[:])
                po = ps_misc.tile([128, D], F32, tag="po", name=f"po{s}")
                nc.tensor.matmul(
                    po[:, :], eT[:V, :], vals_sb[:V, s, :], start=True, stop=True
                )
                o_sb = smp.tile([128, D], F32, tag="o", name=f"o{s}")
                nc.vector.tensor_copy(o_sb[:], po[:])
                nc.sync.dma_start(out_d[s], o_sb[:])
                nc.sync.dma_start(outz_d[s], z[:])

            pending = None
            for s in range(S):
                V = caps[s]
                ngroups = math.ceil(V / KT)
                psc = ps_sc.tile([128, K], F32, tag="psc", name=f"psc{s}")
                prev_last = None
                for g in range(ngroups):
                    k0 = g * KT
                    Kg = min(KT, V - k0)
                    nflat = Kg * Q
                    pre = stagep.tile([128, 4, KT * Q], BF16, tag="pre")
                    tnh = stagep.tile([128, 4, KT * Q], BF16, tag="tnh")
                    for hc in range(4):
                        # pre[h, kl, qp, j] = kproj[h, k0+kl] + qproj[h, 2qp+j]
                        in0 = (
                            kproj2[:, s, hc, k0 : k0 + Kg, :]
                            .unsqueeze(2)
                            .broadcast_to((128, Kg, Q // 2, 2))
                        )
                        in1 = (
                            qproj[:, s, hc, :]
                            .rearrange("p (qp j) -> p qp j", j=2)
                            .unsqueeze(1)
                            .broadcast_to((128, Kg, Q // 2, 2))
                        )
                        out = pre[:, hc, :nflat].rearrange(
                            "p (kl qp j) -> p kl qp j", qp=Q // 2, j=2
                        )
                        nc.vector.tensor_add(out, in0, in1)
                    nc.scalar.activation(
                        tnh[:, :, :nflat], pre[:, :, :nflat], Tanh
                    )
                    tnh3 = tnh[:, :, :nflat].rearrange(
                        "p hc (kl q) -> p hc kl q", q=Q
                    )
                    for kl in range(Kg):
                        first = None
                        for hc in range(4):
                            bi = nc.tensor.matmul(
                                psc[:, k0 + kl : k0 + kl + 1],
                                tnh3[:, hc, kl, :],
                                wv_sb[:, hc : hc + 1],
                                start=(hc == 0),
                                stop=(hc == 3),
                            )
                            if hc == 0:
                                first = bi.ins
                            last = bi.ins
                        if prev_last is not None:
                            tile.add_dep_helper(
                                first, prev_last, sync=False,
                                reason="psc accumulation-group order",
                            )
                        prev_last = last
                if pending is not None:
                    epilogue(*pending)
                pending = (s, psc)
            epilogue(*pending)

    nc.finalize()
    return nc


def kernel(queries, keys, values, valid_lens, Wq, Wk, wv):
    global LAST_RESULT
    queries = np.asarray(queries, dtype=np.float32)
    keys = np.asarray(keys, dtype=np.float32)
    values = np.asarray(values, dtype=np.float32)
    valid_lens = np.asarray(valid_lens, dtype=np.int32)
    Wq = np.asarray(Wq, dtype=np.float32)
    Wk = np.asarray(Wk, dtype=np.float32)
    wv = np.asarray(wv, dtype=np.float32)

    caps, content = _plan(valid_lens)
    S = len(caps)

    if caps not in _NC_CACHE:
        _NC_CACHE[caps] = _build_nc(caps)
    nc = _NC_CACHE[caps]

    # ---- host-side shard prep -------------------------------------------
    wq_bf = Wq.astype(BF16_NP)
    wk_bf = Wk.astype(BF16_NP)
    wv4 = np.ascontiguousarray(wv.reshape(4, 128).T).astype(BF16_NP)  # [128,4]
    qTt = {
        b: np.ascontiguousarray(queries[b].T).astype(BF16_NP) for b in range(B)
    }

    in_maps = []
    for c in range(NCORES):
        qTm = np.zeros((S, D, Q), dtype=BF16_NP)
        kTm = np.zeros((S, D, K), dtype=BF16_NP)
        valsm = np.zeros((S, K, D), dtype=BF16_NP)
        maskm = np.zeros((S, 128, K), dtype=np.float32)
        for s, (b, k0, klen) in enumerate(content[c]):
            if b < 0:
                maskm[s, :, :] = NEG
                continue
            qTm[s] = qTt[b]
            kTm[s, :, :klen] = keys[b, k0 : k0 + klen].T.astype(BF16_NP)
            valsm[s, :klen] = values[b, k0 : k0 + klen].astype(BF16_NP)
            maskm[s, :, klen:] = NEG
        in_maps.append(
            {
                "qT": qTm,
                "kT": kTm,
                "vals": valsm,
                "wq": wq_bf,
                "wk": wk_bf,
                "wv4": wv4,
                "mask": maskm,
            }
        )

    res = run_bass_kernel_spmd(nc, in_maps, list(range(NCORES)))
    LAST_RESULT = res

    O = np.zeros((B, Q, D), dtype=np.float64)
    Z = np.zeros((B, Q, 1), dtype=np.float64)
    for c in range(NCORES):
        o = np.asarray(res.results[c]["out"], dtype=np.float64)
        z = np.asarray(res.results[c]["outz"], dtype=np.float64)
        for s, (b, k0, klen) in enumerate(content[c]):
            if b < 0:
                continue
            O[b] += o[s]
            Z[b] += z[s]
    return (O / Z).astype(np.float32)
